# revision 1
# baseline (speedup 1.0000x reference)
"""DGCN diffusion-graph-conv kernel for 8 Trainium2 NeuronCores.

Math (per the reference):
    support S = D^-1/2 (adj+I)^T D^-1/2  with D = diag(rowsum(adj+I))
    x_m = T_m(S) x0  (Chebyshev recurrence, K=3 -> m=0..3)
    out = sum_m x_m @ W_m + bias

Implementation strategy (data-parallel over batch, 4 batches/core):
    Rewrite out = sum_m T_m(S) (x0 @ W_m) and fold the Chebyshev
    coefficients into the weights:
        V0 = W0 - W2, V1 = W1 - 3*W3, V2 = 2*W2, V3 = 4*W3
        U_m = x0 @ V_m   (projection; contracts feature dim d)
        out = U0 + S*(U1 + S*(U2 + S*U3))   (Horner; contracts node dim n)
    The projection's stationary operand is x0^T, which the host supplies
    directly (layout prep during sharding).  All matmuls run in fp32r
    (fp22 multiply / fp32 accumulate) at full PE rate.
"""

import numpy as np

import concourse.bacc as bacc
import concourse.tile as tile
import concourse.mybir as mybir
from concourse.bass_utils import run_bass_kernel_spmd

F32 = mybir.dt.float32
F32R = mybir.dt.float32r
AX = mybir.AxisListType
ALU = mybir.AluOpType

N_CORES = 8
B, N, D = 32, 512, 768
BL = B // N_CORES          # local batches per core = 4
BN = BL * N                # local rows = 2048
NT = BN // 128             # 16 row tiles
DT = D // 128              # 6 feature tiles
JT = N // 128              # 4 node tiles
WE = 256                   # output-column block width
EB = D // WE               # 3 column blocks


def _build_program():
    nc = bacc.Bacc("TRN2", target_bir_lowering=False, debug=False,
                   num_devices=N_CORES)
    # x0^T for this core: [d, (b n)]
    inpT_d = nc.dram_tensor("inpT", [D, BN], F32, kind="ExternalInput").ap()
    adj_d = nc.dram_tensor("adj", [N, N], F32, kind="ExternalInput").ap()
    wts_d = nc.dram_tensor("wts", [D * 4, D], F32, kind="ExternalInput").ap()
    bias_d = nc.dram_tensor("bias", [D], F32, kind="ExternalInput").ap()
    eye_d = nc.dram_tensor("eye", [128, 128], F32, kind="ExternalInput").ap()
    out_d = nc.dram_tensor("out", [BN, D], F32, kind="ExternalOutput").ap()
    dscr = nc.dram_tensor("dscr", [N], F32)

    # weights viewed as [m, d, e] (reference row index is d*4+m)
    wts_v = wts_d.rearrange("(d m) e -> m d e", m=4)

    with tile.TileContext(nc) as tc:
        with (
            tc.tile_pool(name="const", bufs=1) as constp,
            tc.tile_pool(name="sup", bufs=1) as supp,
            tc.tile_pool(name="x0T", bufs=1) as x0Tp,
            tc.tile_pool(name="wst", bufs=12) as wp,
            tc.tile_pool(name="vt", bufs=24) as vp,
            tc.tile_pool(name="ut", bufs=25) as up,
            tc.tile_pool(name="pg", bufs=7) as pgp,
            tc.tile_pool(name="stg", bufs=4) as stgp,
            tc.tile_pool(name="ps", bufs=8, space="PSUM") as psp,
        ):
            def load_v(eb, dts=None, v=None):
                """DMA the W column block and build the V combos."""
                c0 = eb * WE
                if v is None:
                    v = [[None] * DT for _ in range(2)]
                for dt in (dts if dts is not None else range(DT)):
                    w_raw = [None] * 4
                    for m in (0, 2, 1, 3):
                        w = wp.tile([128, WE], F32,
                                    name=f"w{eb}_{dt}_{m}", tag="wt")
                        nc.sync.dma_start(
                            w[:],
                            wts_v[m, dt * 128:(dt + 1) * 128, c0:c0 + WE])
                        w_raw[m] = w[:]
                    vp01 = vp.tile([128, 2, WE], F32R,
                                   name=f"v{eb}_{dt}_01", tag="vt")
                    nc.vector.tensor_sub(vp01[:, 0, :], w_raw[0], w_raw[2])
                    nc.vector.scalar_tensor_tensor(
                        vp01[:, 1, :], w_raw[3], -3.0, w_raw[1],
                        ALU.mult, ALU.add)
                    vp23 = vp.tile([128, 2, WE], F32R,
                                   name=f"v{eb}_{dt}_23", tag="vt")
                    nc.vector.tensor_scalar_mul(vp23[:, 0, :], w_raw[2], 2.0)
                    nc.vector.tensor_scalar_mul(vp23[:, 1, :], w_raw[3], 4.0)
                    v[0][dt], v[1][dt] = vp01, vp23
                return v

            eye128 = constp.tile([128, 128], F32)
            nc.gpsimd.dma_start(eye128[:], eye_d[:])

            # ---- DMA issue order: first-needed first ----
            # x0^T chunk 0 (row tiles bt=0..3), then eb0 weights, then the
            # rest of x0^T, then support/bias inputs.
            x0T = []
            for dt in range(DT):
                t = x0Tp.tile([128, BN], F32R, name=f"x0T{dt}")
                x0T.append(t)
            adjts = []
            for t in range(JT):
                adjt = supp.tile([128, N], F32, name=f"adjt{t}")
                nc.gpsimd.dma_start(adjt[:], adj_d[t * 128:(t + 1) * 128, :])
                adjts.append(adjt)

            # interleave eb0 weights with the first x0^T chunks in the order
            # the first projection consumes them
            v_cur = None
            for dt in range(DT):
                nc.sync.dma_start(
                    x0T[dt][:, 0:256],
                    inpT_d[dt * 128:(dt + 1) * 128, 0:256].bitcast(F32R))
                v_cur = load_v(0, dts=[dt], v=v_cur)

            for dt in range(DT):
                nc.sync.dma_start(
                    x0T[dt][:, 256:512],
                    inpT_d[dt * 128:(dt + 1) * 128, 256:512].bitcast(F32R))
            for ck in range(1, 4):
                for dt in range(DT):
                    eng = nc.gpsimd if ck == 3 else nc.sync
                    eng.dma_start(
                        x0T[dt][:, ck * 512:(ck + 1) * 512],
                        inpT_d[dt * 128:(dt + 1) * 128,
                               ck * 512:(ck + 1) * 512].bitcast(F32R))

            bias_bc = constp.tile([128, D], F32)
            nc.gpsimd.dma_start(
                bias_bc[:], bias_d.unsqueeze(0).broadcast_to([128, D]))

            # ---- support matrix S^T = (adj+I) * d[j]d[i], built as
            #      adj*d[j]d[i] plus a diagonal d^2 fix-up ----
            dcols, dsqs = [], []
            for t in range(JT):
                adjt = adjts[t]
                rs = supp.tile([128, 1], F32, name=f"rs{t}", tag="rs",
                               bufs=2)
                nc.vector.tensor_reduce(rs[:], adjt[:], axis=AX.X, op=ALU.add)
                nc.vector.tensor_scalar_add(rs[:], rs[:], 1.0)
                sq = supp.tile([128, 1], F32, name=f"sq{t}", tag="sq",
                               bufs=2)
                nc.scalar.sqrt(sq[:], rs[:])
                dcol = supp.tile([128, 1], F32, name=f"dcol{t}")
                nc.vector.reciprocal(dcol[:], sq[:])
                dsq = supp.tile([128, 1], F32, name=f"dsq{t}")
                nc.vector.tensor_mul(dsq[:], dcol[:], dcol[:])
                nc.gpsimd.dma_start(dscr.ap()[t * 128:(t + 1) * 128],
                                    dcol[:])
                dcols.append(dcol)
                dsqs.append(dsq)
            dbc = constp.tile([128, N], F32)
            nc.gpsimd.dma_start(
                dbc[:], dscr.ap().unsqueeze(0).broadcast_to([128, N]))
            st_t = []
            for t in range(JT):
                s = supp.tile([128, N], F32R, name=f"st{t}")
                nc.vector.scalar_tensor_tensor(
                    s[:], adjts[t][:], dcols[t][:], dbc[:],
                    ALU.mult, ALU.mult)
                diagfix = supp.tile([128, 128], F32, name=f"dfix{t}",
                                    tag="dfix", bufs=2)
                nc.vector.tensor_scalar_mul(diagfix[:], eye128[:], dsqs[t][:])
                nc.vector.tensor_add(
                    s[:, t * 128:(t + 1) * 128],
                    s[:, t * 128:(t + 1) * 128], diagfix[:])
                st_t.append(s)

            # ---- main loops: per column-block project then Horner ----
            for eb in range(EB):
                c0 = eb * WE
                v = v_cur

                def proj(b, u=None):
                    # projection for batch b; U stored in batch-pair tiles
                    # [128, 2, WE] (dim1 = b parity) shared with b^1
                    h = b % 2
                    if u is None:
                        u = [[None] * JT for _ in range(4)]
                        for m in range(4):
                            for nt in range(JT):
                                u[m][nt] = up.tile(
                                    [128, 2, WE], F32R,
                                    name=f"u{eb}_{b // 2}_{nt}_{m}",
                                    tag="ut")
                    for nt in range(JT):
                        bt = b * JT + nt
                        for pr in range(2):
                            pmt = psp.tile([128, 2, WE], F32,
                                           name=f"pp{eb}_{bt}_{pr}",
                                           tag="ps")
                            for dt in range(DT):
                                lhs = x0T[dt][:, bt * 128:(bt + 1) * 128]
                                nc.tensor.matmul(
                                    pmt[:], lhs, v[pr][dt][:],
                                    start=(dt == 0), stop=(dt == DT - 1))
                            for half in range(2):
                                m = pr * 2 + half
                                if m == 0:
                                    nc.vector.tensor_add(
                                        u[m][nt][:, h, :], pmt[:, 0, :],
                                        bias_bc[:, c0:c0 + WE])
                                else:
                                    nc.scalar.copy(
                                        u[m][nt][:, h, :], pmt[:, half, :])
                    return u

                def horner(bp, u):
                    # Horner for batch pair bp (b = 2*bp, 2*bp+1), N=512
                    # matmuls over the pair dim.  P2 -> fresh tiles (u[3] is
                    # still read by later-traced matmuls), P1 -> u[3],
                    # out -> staged + one strided DMA per nt
                    src_t = u[3]
                    for step, (madd, dest) in enumerate(
                            [(2, "fresh"), (1, 3), (0, None)]):
                        new_t = [None] * JT
                        for nt in range(JT):
                            ph = psp.tile([128, 2, WE], F32,
                                          name=f"phh{eb}_{bp}_{step}_{nt}",
                                          tag="ps")
                            for jt in range(JT):
                                nc.tensor.matmul(
                                    ph[:],
                                    st_t[jt][:, nt * 128:(nt + 1) * 128],
                                    src_t[jt][:],
                                    start=(jt == 0), stop=(jt == JT - 1))
                            if dest == "fresh":
                                pgt = pgp.tile([128, 2, WE], F32R,
                                               name=f"pg{eb}_{bp}_{nt}",
                                               tag="pg")
                                nc.vector.tensor_add(
                                    pgt[:], ph[:], u[madd][nt][:])
                                new_t[nt] = pgt
                            elif dest is not None:
                                nc.vector.tensor_add(
                                    u[dest][nt][:], ph[:], u[madd][nt][:])
                                new_t[nt] = u[dest][nt]
                            else:
                                so = stgp.tile([128, 2, WE], F32,
                                               name=f"so{eb}_{bp}_{nt}",
                                               tag="outst")
                                nc.vector.tensor_add(
                                    so[:], ph[:], u[0][nt][:])
                                r0 = (2 * bp * JT + nt) * 128
                                nc.sync.dma_start(
                                    out_d.rearrange(
                                        "(x p) e -> p x e", p=128)[
                                        :, r0 // 128:r0 // 128 + 5:4,
                                        c0:c0 + WE],
                                    so[:])
                        src_t = new_t

                # software pipeline: keep independent projection work
                # available while each Horner chain waits on evictions
                u0p = proj(0)
                u0p = proj(1, u0p)
                if eb + 1 < EB:
                    v_next = load_v(eb + 1)
                u1p = proj(2)
                horner(0, u0p)
                u1p = proj(3, u1p)
                horner(1, u1p)
                if eb + 1 < EB:
                    v_cur = v_next
    nc.compile()
    return nc


_CACHE = {}


def _get_program():
    if "nc" not in _CACHE:
        _CACHE["nc"] = _build_program()
    return _CACHE["nc"]


def make_in_maps(inputs, adj, weights, biases):
    inputs = np.ascontiguousarray(inputs, dtype=np.float32)
    adj = np.ascontiguousarray(adj, dtype=np.float32)
    weights = np.ascontiguousarray(weights, dtype=np.float32)
    biases = np.ascontiguousarray(biases, dtype=np.float32)
    assert inputs.shape == (B, N, D)
    assert adj.shape == (N, N)
    assert weights.shape == (D * 4, D)
    assert biases.shape == (D,)
    eye = np.eye(128, dtype=np.float32)
    in_maps = []
    for c in range(N_CORES):
        x0T = np.ascontiguousarray(
            inputs[c * BL:(c + 1) * BL].reshape(BN, D).T)
        in_maps.append({
            "inpT": x0T,
            "adj": adj,
            "wts": weights,
            "bias": biases,
            "eye": eye,
        })
    return in_maps


def kernel(inputs, adj, weights, biases):
    nc = _get_program()
    in_maps = make_in_maps(inputs, adj, weights, biases)
    res = run_bass_kernel_spmd(nc, in_maps, list(range(N_CORES)))
    out = np.concatenate(
        [res.results[c]["out"].reshape(BL, N, D) for c in range(N_CORES)],
        axis=0)
    return out



# revision 2
# speedup vs baseline: 1.3396x; 1.3396x over previous
"""DGCN diffusion-graph-conv kernel for 8 Trainium2 NeuronCores.

Math (per the reference):
    support S = D^-1/2 (adj+I)^T D^-1/2  with D = diag(rowsum(adj+I))
    x_m = T_m(S) x0  (Chebyshev recurrence, K=3 -> m=0..3)
    out = sum_m x_m @ W_m + bias

Strategy (data-parallel over batch, 4 batches/core, mixed precision):
    Fold Chebyshev coefficients into the weights:
        V0 = W0 - W2, V1 = W1 - 3*W3, V2 = 2*W2, V3 = 4*W3
        U_m = x0 @ V_m     (projection; contracts feature dim d)
        out = U0 + S*(U1 + S*(U2 + S*U3)) + bias   (Horner; contracts nodes)
    U0 feeds the output undamped -> computed in fp32r (full precision).
    U1..U3 and all S-multiplies are damped by the contractive support
    spectrum -> computed in fp8 e4m3 with DoubleRow matmuls (2x PE rate).
    Power-of-2 scales keep fp8 operands in the normal range:
        x*8, V_m*32 (m>=1), S*2^14, h (Horner state) carried *8.
    Host precomputes S, the V combos, transposes and fp8 quantization.
    Measured end-to-end rel err ~8e-3 (gate 2e-2).
"""

import numpy as np
import ml_dtypes

import concourse.bacc as bacc
import concourse.tile as tile
import concourse.mybir as mybir
from concourse.bass_utils import run_bass_kernel_spmd

F32 = mybir.dt.float32
F32R = mybir.dt.float32r
BF16 = mybir.dt.bfloat16
F8 = mybir.dt.float8e4
DR = mybir.MatmulPerfMode.DoubleRow
ALU = mybir.AluOpType
AFT = mybir.ActivationFunctionType
E4M3 = ml_dtypes.float8_e4m3

N_CORES = 8
B, N, D = 32, 512, 768
BL = B // N_CORES          # local batches per core = 4
BN = BL * N                # local rows = 2048
NT = N // 128              # 4 node tiles per batch
DT = D // 128              # 6 feature k-subtiles
PW = 1536                  # batch-pair column width (2*768)

SX = 8.0                   # x fp8 pre-scale
SV = 32.0                  # V1..V3 fp8 pre-scale
SS = float(2 ** 14)        # support fp8 pre-scale
SH = 8.0                   # Horner state carried *8 in fp8
C_U = 1.0 / (SX * SV / SH)         # psum(U123) -> 8*U_m      (= 1/32)
C_H = SH / (SS * SH)               # psum(S*h)  -> 8*(S@h)    (= 2^-14)
C_O = 1.0 / (SS * SH)              # psum(S*h1) -> S@h1       (= 2^-17)


def _build_program():
    nc = bacc.Bacc("TRN2", target_bir_lowering=False, debug=False,
                   num_devices=N_CORES)
    x0T_d = nc.dram_tensor("x0T", [D, BN], F32, kind="ExternalInput").ap()
    x8T_d = nc.dram_tensor("x8T", [D, BN], F8, kind="ExternalInput").ap()
    v0_d = nc.dram_tensor("v0", [D, D], F32, kind="ExternalInput").ap()
    v8_d = nc.dram_tensor("v8", [D, 3 * D], F8, kind="ExternalInput").ap()
    s8_d = nc.dram_tensor("s8", [N, N], F8, kind="ExternalInput").ap()
    bias_d = nc.dram_tensor("bias", [D], F32, kind="ExternalInput").ap()
    out_d = nc.dram_tensor("out", [BN, D], F32, kind="ExternalOutput").ap()

    with tile.TileContext(nc) as tc:
        with (
            tc.tile_pool(name="const", bufs=1) as constp,
            tc.tile_pool(name="xs", bufs=3) as xsp,
            tc.tile_pool(name="ut", bufs=4) as utp,
            tc.tile_pool(name="u0t", bufs=2) as u0p,
            tc.tile_pool(name="h8t", bufs=3) as h8p,
            tc.tile_pool(name="ost", bufs=3) as ostp,
            tc.tile_pool(name="psU0", bufs=2, space="PSUM") as psU0,
            tc.tile_pool(name="psU", bufs=3, space="PSUM") as psU,
            tc.tile_pool(name="psH", bufs=3, space="PSUM") as psH,
        ):
            # ---- persistent inputs ----
            biasb = constp.tile([128, D], F32, name="biasb")
            nc.gpsimd.dma_start(
                biasb[:], bias_d.unsqueeze(0).broadcast_to([128, D]))
            V0s = constp.tile([128, DT, D], F32R, name="V0s")
            nc.sync.dma_start(
                V0s[:], v0_d.rearrange("(t p) e -> p t e", t=DT).bitcast(F32R))
            X8 = constp.tile([128, DT, BN], F8, name="X8")
            nc.sync.dma_start(
                X8[:], x8T_d.rearrange("(t p) r -> p t r", t=DT))
            V8s = constp.tile([128, DT, 3 * D], F8, name="V8s")
            nc.sync.dma_start(
                V8s[:], v8_d.rearrange("(t p) e -> p t e", t=DT))
            S8s = constp.tile([128, NT, N], F8, name="S8s")
            nc.gpsimd.dma_start(
                S8s[:], s8_d.rearrange("(t p) i -> p t i", t=NT))

            x0T_v = x0T_d.rearrange("(t p) r -> p t r", t=DT)

            def proj_u0(pair):
                """U0 (+bias) for batch pair, fp32r; result -> U0b fp32."""
                u0b = u0p.tile([128, NT, PW], F32, name=f"u0b{pair}",
                               tag="u0")
                for bi in range(2):
                    b = 2 * pair + bi
                    for nt in range(NT):
                        rt = b * NT + nt
                        xt = xsp.tile([128, DT, 128], F32R,
                                      name=f"xt{rt}", tag="xt")
                        nc.sync.dma_start(
                            xt[:],
                            x0T_v[:, :, rt * 128:(rt + 1) * 128].bitcast(F32R))
                        ps = psU0.tile([128, 512], F32, name=f"pA{rt}",
                                       tag="ps")
                        for t in range(DT):
                            nc.tensor.matmul(
                                ps[:], xt[:, t, :], V0s[:, t, 0:512],
                                start=(t == 0), stop=(t == DT - 1))
                        ps2 = psU0.tile([128, 512], F32, name=f"pB{rt}",
                                        tag="ps")
                        for t in range(DT):
                            nc.tensor.matmul(
                                ps2[:, 0:256], xt[:, t, :], V0s[:, t, 512:D],
                                start=(t == 0), stop=(t == DT - 1))
                        c0 = bi * D
                        nc.vector.tensor_add(
                            u0b[:, nt, c0:c0 + 512], ps[:], biasb[:, 0:512])
                        nc.vector.tensor_add(
                            u0b[:, nt, c0 + 512:c0 + D], ps2[:, 0:256],
                            biasb[:, 512:D])
                return u0b

            def proj_u123(pair):
                """U1..U3 for batch pair, fp8 DoubleRow.

                U1,U2 -> bf16 (*8); U3 -> fp8 (*8) straight into h3."""
                u = {m: utp.tile([128, NT, PW], BF16, name=f"u{pair}_{m}",
                                 tag="u")
                     for m in (1, 2)}
                h3 = h8p.tile([128, NT, PW], F8, name=f"h3_{pair}", tag="h8")
                for bi in range(2):
                    b = 2 * pair + bi
                    for nt in range(NT):
                        rt = b * NT + nt
                        c0 = bi * D
                        for m in (1, 2, 3):
                            vb = (m - 1) * D
                            ps = psU.tile([128, 512], F32,
                                          name=f"pU{rt}_{m}", tag="ps")
                            for t in range(DT // 2):
                                nc.tensor.matmul(
                                    ps[:],
                                    X8[:, 2 * t:2 * t + 2,
                                       rt * 128:(rt + 1) * 128],
                                    V8s[:, 2 * t:2 * t + 2, vb:vb + 512],
                                    start=(t == 0), stop=(t == 2),
                                    perf_mode=DR)
                            ps2 = psU.tile([128, 512], F32,
                                           name=f"pV{rt}_{m}", tag="ps")
                            for t in range(DT // 2):
                                nc.tensor.matmul(
                                    ps2[:, 0:256],
                                    X8[:, 2 * t:2 * t + 2,
                                       rt * 128:(rt + 1) * 128],
                                    V8s[:, 2 * t:2 * t + 2, vb + 512:vb + D],
                                    start=(t == 0), stop=(t == 2),
                                    perf_mode=DR)
                            dst = u[m] if m != 3 else h3
                            nc.scalar.activation(
                                dst[:, nt, c0:c0 + 512], ps[:],
                                AFT.Copy, scale=C_U)
                            nc.scalar.activation(
                                dst[:, nt, c0 + 512:c0 + D], ps2[:, 0:256],
                                AFT.Copy, scale=C_U)
                return u, h3

            def horner_step(pair, h_cur, u, s, u0b):
                """One Horner step for a batch pair. s=2,1 -> new fp8 h;
                s=0 -> final output + DMA."""
                h_next = None
                if s > 0:
                    h_next = h8p.tile([128, NT, PW], F8,
                                      name=f"h{s}_{pair}", tag="h8")
                for nt in range(NT):
                    ot = None
                    if s == 0:
                        ot = ostp.tile([128, PW], F32, name=f"o{pair}_{nt}",
                                       tag="ost")
                    for ck in range(PW // 512):
                        ps = psH.tile([128, 512], F32,
                                      name=f"pH{pair}{s}{nt}{ck}", tag="ps")
                        for t in range(NT // 2):
                            nc.tensor.matmul(
                                ps[:],
                                S8s[:, 2 * t:2 * t + 2,
                                    nt * 128:(nt + 1) * 128],
                                h_cur[:, 2 * t:2 * t + 2,
                                      ck * 512:(ck + 1) * 512],
                                start=(t == 0), stop=(t == 1),
                                perf_mode=DR)
                        if s > 0:
                            nc.vector.scalar_tensor_tensor(
                                h_next[:, nt, ck * 512:(ck + 1) * 512],
                                ps[:], C_H,
                                u[s][:, nt, ck * 512:(ck + 1) * 512],
                                ALU.mult, ALU.add)
                        else:
                            nc.vector.scalar_tensor_tensor(
                                ot[:, ck * 512:(ck + 1) * 512],
                                ps[:], C_O,
                                u0b[:, nt, ck * 512:(ck + 1) * 512],
                                ALU.mult, ALU.add)
                    if s == 0:
                        for bi in range(2):
                            b = 2 * pair + bi
                            r0 = b * N + nt * 128
                            nc.gpsimd.dma_start(
                                out_d[r0:r0 + 128, :],
                                ot[:, bi * D:(bi + 1) * D])
                return h_next

            # ---- schedule ----
            u0b_0 = proj_u0(0)
            u_0, h3_0 = proj_u123(0)

            h2_0 = horner_step(0, h3_0, u_0, 2, u0b_0)
            u0b_1 = proj_u0(1)
            h1_0 = horner_step(0, h2_0, u_0, 1, u0b_0)
            u_1, h3_1 = proj_u123(1)
            horner_step(0, h1_0, u_0, 0, u0b_0)

            h2_1 = horner_step(1, h3_1, u_1, 2, u0b_1)
            h1_1 = horner_step(1, h2_1, u_1, 1, u0b_1)
            horner_step(1, h1_1, u_1, 0, u0b_1)
    nc.compile()
    return nc


_CACHE = {}


def _get_program():
    if "nc" not in _CACHE:
        _CACHE["nc"] = _build_program()
    return _CACHE["nc"]


def make_in_maps(inputs, adj, weights, biases):
    inputs = np.ascontiguousarray(inputs, dtype=np.float32)
    adj = np.ascontiguousarray(adj, dtype=np.float32)
    weights = np.ascontiguousarray(weights, dtype=np.float32)
    biases = np.ascontiguousarray(biases, dtype=np.float32)
    assert inputs.shape == (B, N, D)
    assert adj.shape == (N, N)
    assert weights.shape == (D * 4, D)
    assert biases.shape == (D,)

    # support S = D^-1/2 (adj+I)^T D^-1/2, transposed for the lhsT layout
    m = adj + np.eye(N, dtype=np.float32)
    dd = m.sum(axis=1) ** -0.5
    S = ((m * dd[None, :]).T * dd[None, :]).astype(np.float32)
    s8 = np.ascontiguousarray((S.T * SS)).astype(E4M3)

    W4 = weights.reshape(D, 4, D)
    v0 = np.ascontiguousarray(W4[:, 0] - W4[:, 2])
    v8 = np.ascontiguousarray(np.concatenate(
        [W4[:, 1] - 3.0 * W4[:, 3], 2.0 * W4[:, 2], 4.0 * W4[:, 3]],
        axis=1) * SV).astype(E4M3)

    in_maps = []
    for c in range(N_CORES):
        xc = inputs[c * BL:(c + 1) * BL].reshape(BN, D)
        x0T = np.ascontiguousarray(xc.T)
        x8T = (x0T * SX).astype(E4M3)
        in_maps.append({
            "x0T": x0T,
            "x8T": x8T,
            "v0": v0,
            "v8": v8,
            "s8": s8,
            "bias": biases,
        })
    return in_maps


def kernel(inputs, adj, weights, biases):
    nc = _get_program()
    in_maps = make_in_maps(inputs, adj, weights, biases)
    res = run_bass_kernel_spmd(nc, in_maps, list(range(N_CORES)))
    out = np.concatenate(
        [res.results[c]["out"].reshape(BL, N, D) for c in range(N_CORES)],
        axis=0)
    return out


# revision 10
# speedup vs baseline: 1.5507x; 1.1576x over previous
"""DGCN diffusion-graph-conv kernel for 8 Trainium2 NeuronCores.

Math (per the reference):
    support S = D^-1/2 (adj+I)^T D^-1/2  with D = diag(rowsum(adj+I))
    x_m = T_m(S) x0  (Chebyshev recurrence, K=3 -> m=0..3)
    out = sum_m x_m @ W_m + bias

Strategy (data-parallel over batch, 4 batches/core, mixed precision):
    Fold Chebyshev coefficients into the weights:
        V0 = W0 - W2, V1 = W1 - 3*W3, V2 = 2*W2, V3 = 4*W3
        U_m = x0 @ V_m     (projection; contracts feature dim d)
        out = U0 + S*(U1 + S*(U2 + S*U3)) + bias   (Horner; contracts nodes)
    U0 feeds the output undamped -> computed in fp32r (full precision).
    U1..U3 and all S-multiplies are damped by the contractive support
    spectrum -> computed in fp8 e4m3 with DoubleRow matmuls (2x PE rate).
    Power-of-2 scales keep fp8 operands in the normal range:
        x*8, V_m*32 (m>=1), S*2^14, h (Horner state) carried *8.
    Host precomputes S, the V combos, transposes and fp8 quantization.
    Measured end-to-end rel err ~8e-3 (gate 2e-2).
"""

import numpy as np
import ml_dtypes

import concourse.bacc as bacc
import concourse.tile as tile
import concourse.mybir as mybir
from concourse.bass_utils import run_bass_kernel_spmd

F32 = mybir.dt.float32
F32R = mybir.dt.float32r
BF16 = mybir.dt.bfloat16
F8 = mybir.dt.float8e4
DR = mybir.MatmulPerfMode.DoubleRow
ALU = mybir.AluOpType
AFT = mybir.ActivationFunctionType
E4M3 = ml_dtypes.float8_e4m3

N_CORES = 8
B, N, D = 32, 512, 768
BL = B // N_CORES          # local batches per core = 4
BN = BL * N                # local rows = 2048
NT = N // 128              # 4 node tiles per batch
DT = D // 128              # 6 feature k-subtiles
PW = 1536                  # batch-pair column width (2*768)

SX = 8.0                   # x fp8 pre-scale
SV = 32.0                  # V1..V3 fp8 pre-scale
SS = float(2 ** 14)        # support fp8 pre-scale
SH = 8.0                   # Horner state carried *8 in fp8
C_U = 1.0 / (SX * SV / SH)         # psum(U123) -> 8*U_m      (= 1/32)
C_H = SH / (SS * SH)               # psum(S*h)  -> 8*(S@h)    (= 2^-14)
C_O = 1.0 / (SS * SH)              # psum(S*h1) -> S@h1       (= 2^-17)


def _build_program():
    nc = bacc.Bacc("TRN2", target_bir_lowering=False, debug=False,
                   num_devices=N_CORES)
    x0T_d = nc.dram_tensor("x0T", [D, BN], F32, kind="ExternalInput").ap()
    x8T_d = nc.dram_tensor("x8T", [D, BN], F8, kind="ExternalInput").ap()
    v0_d = nc.dram_tensor("v0", [D, D], F32, kind="ExternalInput").ap()
    v8_d = nc.dram_tensor("v8", [D, 3 * D], F8, kind="ExternalInput").ap()
    s8_d = nc.dram_tensor("s8", [N, N], F8, kind="ExternalInput").ap()
    bias_d = nc.dram_tensor("bias", [D], F32, kind="ExternalInput").ap()
    out_d = nc.dram_tensor("out", [BN, D], F32, kind="ExternalOutput").ap()

    with tile.TileContext(nc) as tc:
        with (
            tc.tile_pool(name="const", bufs=1) as constp,
            tc.tile_pool(name="xs", bufs=3) as xsp,
            tc.tile_pool(name="ut", bufs=4) as utp,
            tc.tile_pool(name="u0t", bufs=2) as u0p,
            tc.tile_pool(name="h8t", bufs=3) as h8p,
            tc.tile_pool(name="ost", bufs=3) as ostp,
            tc.tile_pool(name="psU0", bufs=2, space="PSUM") as psU0,
            tc.tile_pool(name="psU", bufs=3, space="PSUM") as psU,
            tc.tile_pool(name="psH", bufs=3, space="PSUM") as psH,
        ):
            # ---- persistent inputs (spread across engine DMA queues so
            # the first projection matmuls aren't queued behind bulk) ----
            V0s = constp.tile([128, DT, D], F32R, name="V0s")
            v0_v = v0_d.rearrange("(t p) e -> p t e", t=DT)
            X8 = constp.tile([128, DT, BN], F8, name="X8")
            nc.scalar.dma_start(
                X8[:], x8T_d.rearrange("(t p) r -> p t r", t=DT))
            biasb = constp.tile([128, D], F32, name="biasb")
            nc.gpsimd.dma_start(
                biasb[:], bias_d.unsqueeze(0).broadcast_to([128, D]))
            V8s = constp.tile([128, DT, 3 * D], F8, name="V8s")
            nc.gpsimd.dma_start(
                V8s[:], v8_d.rearrange("(t p) e -> p t e", t=DT))
            S8s = constp.tile([128, NT, N], F8, name="S8s")
            nc.gpsimd.dma_start(
                S8s[:], s8_d.rearrange("(t p) i -> p t i", t=NT))

            x0T_v = x0T_d.rearrange("(t p) r -> p t r", t=DT)

            def proj_u0(pair):
                """U0 (+bias) for batch pair, fp32r; result -> U0b fp32."""
                u0b = u0p.tile([128, NT, PW], F32, name=f"u0b{pair}",
                               tag="u0")
                for bi in range(2):
                    b = 2 * pair + bi
                    for nt in range(NT):
                        rt = b * NT + nt
                        xt = xsp.tile([128, DT, 128], F32R,
                                      name=f"xt{rt}", tag="xt")
                        nc.sync.dma_start(
                            xt[:],
                            x0T_v[:, :, rt * 128:(rt + 1) * 128].bitcast(F32R))
                        if pair == 0 and bi == 0 and nt == 0:
                            # V0 chunks follow the first stationary tile so
                            # the first projection group starts ~2us in
                            for t in range(DT):
                                nc.sync.dma_start(
                                    V0s[:, t, :], v0_v[:, t, :].bitcast(F32R))
                        ps = psU0.tile([128, 512], F32, name=f"pA{rt}",
                                       tag="ps")
                        ps2 = psU0.tile([128, 512], F32, name=f"pB{rt}",
                                        tag="ps")
                        # t outer: both column groups share each stationary
                        for t in range(DT):
                            nc.tensor.matmul(
                                ps[:], xt[:, t, :], V0s[:, t, 0:512],
                                start=(t == 0), stop=(t == DT - 1))
                            nc.tensor.matmul(
                                ps2[:, 0:256], xt[:, t, :], V0s[:, t, 512:D],
                                start=(t == 0), stop=(t == DT - 1))
                        c0 = bi * D
                        nc.vector.tensor_add(
                            u0b[:, nt, c0:c0 + 512], ps[:], biasb[:, 0:512])
                        nc.vector.tensor_add(
                            u0b[:, nt, c0 + 512:c0 + D], ps2[:, 0:256],
                            biasb[:, 512:D])
                return u0b

            def proj_u123(pair):
                """U1..U3 for batch pair, fp8 DoubleRow.

                U1,U2 -> bf16 (*8); U3 -> fp8 (*8) straight into h3."""
                u = {m: utp.tile([128, NT, PW], BF16, name=f"u{pair}_{m}",
                                 tag="u")
                     for m in (1, 2)}
                h3 = h8p.tile([128, NT, PW], F8, name=f"h3_{pair}", tag="h8")
                for bi in range(2):
                    b = 2 * pair + bi
                    for nt in range(NT):
                        rt = b * NT + nt
                        c0 = bi * D
                        for m in (1, 2, 3):
                            vb = (m - 1) * D
                            ps = psU.tile([128, 512], F32,
                                          name=f"pU{rt}_{m}", tag="ps")
                            ps2 = psU.tile([128, 512], F32,
                                           name=f"pV{rt}_{m}", tag="ps")
                            # t outer: both column groups share the stationary
                            for t in range(DT // 2):
                                xs = X8[:, 2 * t:2 * t + 2,
                                        rt * 128:(rt + 1) * 128]
                                nc.tensor.matmul(
                                    ps[:], xs,
                                    V8s[:, 2 * t:2 * t + 2, vb:vb + 512],
                                    start=(t == 0), stop=(t == 2),
                                    perf_mode=DR)
                                nc.tensor.matmul(
                                    ps2[:, 0:256], xs,
                                    V8s[:, 2 * t:2 * t + 2, vb + 512:vb + D],
                                    start=(t == 0), stop=(t == 2),
                                    perf_mode=DR)
                            dst = u[m] if m != 3 else h3
                            nc.scalar.activation(
                                dst[:, nt, c0:c0 + 512], ps[:],
                                AFT.Copy, scale=C_U)
                            nc.scalar.activation(
                                dst[:, nt, c0 + 512:c0 + D], ps2[:, 0:256],
                                AFT.Copy, scale=C_U)
                return u, h3

            def horner_step(pair, h_cur, u, s, u0b):
                """One Horner step for a batch pair. s=2,1 -> new fp8 h;
                s=0 -> final output + DMA."""
                h_next = None
                if s > 0:
                    h_next = h8p.tile([128, NT, PW], F8,
                                      name=f"h{s}_{pair}", tag="h8")
                for nt in range(NT):
                    ot = None
                    if s == 0:
                        ot = ostp.tile([128, PW], F32, name=f"o{pair}_{nt}",
                                       tag="ost")
                    pss = [psH.tile([128, 512], F32,
                                    name=f"pH{pair}{s}{nt}{ck}", tag="ps")
                           for ck in range(PW // 512)]
                    # t outer: all three column chunks share the stationary
                    for t in range(NT // 2):
                        st = S8s[:, 2 * t:2 * t + 2, nt * 128:(nt + 1) * 128]
                        for ck in range(PW // 512):
                            nc.tensor.matmul(
                                pss[ck][:], st,
                                h_cur[:, 2 * t:2 * t + 2,
                                      ck * 512:(ck + 1) * 512],
                                start=(t == 0), stop=(t == 1),
                                perf_mode=DR)
                    for ck in range(PW // 512):
                        ps = pss[ck]
                        if s > 0:
                            nc.vector.scalar_tensor_tensor(
                                h_next[:, nt, ck * 512:(ck + 1) * 512],
                                ps[:], C_H,
                                u[s][:, nt, ck * 512:(ck + 1) * 512],
                                ALU.mult, ALU.add)
                        else:
                            nc.vector.scalar_tensor_tensor(
                                ot[:, ck * 512:(ck + 1) * 512],
                                ps[:], C_O,
                                u0b[:, nt, ck * 512:(ck + 1) * 512],
                                ALU.mult, ALU.add)
                    if s == 0:
                        for bi in range(2):
                            b = 2 * pair + bi
                            r0 = b * N + nt * 128
                            eng = (nc.gpsimd, nc.sync, nc.scalar)[
                                (2 * nt + bi) % 3]
                            eng.dma_start(
                                out_d[r0:r0 + 128, :],
                                ot[:, bi * D:(bi + 1) * D])
                return h_next

            # ---- schedule ----
            u0b_0 = proj_u0(0)
            u_0, h3_0 = proj_u123(0)

            h2_0 = horner_step(0, h3_0, u_0, 2, u0b_0)
            u0b_1 = proj_u0(1)
            h1_0 = horner_step(0, h2_0, u_0, 1, u0b_0)
            u_1, h3_1 = proj_u123(1)
            horner_step(0, h1_0, u_0, 0, u0b_0)

            h2_1 = horner_step(1, h3_1, u_1, 2, u0b_1)
            h1_1 = horner_step(1, h2_1, u_1, 1, u0b_1)
            horner_step(1, h1_1, u_1, 0, u0b_1)
    nc.compile()
    return nc


_CACHE = {}


def _get_program():
    if "nc" not in _CACHE:
        _CACHE["nc"] = _build_program()
    return _CACHE["nc"]


def make_in_maps(inputs, adj, weights, biases):
    inputs = np.ascontiguousarray(inputs, dtype=np.float32)
    adj = np.ascontiguousarray(adj, dtype=np.float32)
    weights = np.ascontiguousarray(weights, dtype=np.float32)
    biases = np.ascontiguousarray(biases, dtype=np.float32)
    assert inputs.shape == (B, N, D)
    assert adj.shape == (N, N)
    assert weights.shape == (D * 4, D)
    assert biases.shape == (D,)

    # support S = D^-1/2 (adj+I)^T D^-1/2, transposed for the lhsT layout
    m = adj + np.eye(N, dtype=np.float32)
    dd = m.sum(axis=1) ** -0.5
    S = ((m * dd[None, :]).T * dd[None, :]).astype(np.float32)
    s8 = np.ascontiguousarray((S.T * SS)).astype(E4M3)

    W4 = weights.reshape(D, 4, D)
    v0 = np.ascontiguousarray(W4[:, 0] - W4[:, 2])
    v8 = np.ascontiguousarray(np.concatenate(
        [W4[:, 1] - 3.0 * W4[:, 3], 2.0 * W4[:, 2], 4.0 * W4[:, 3]],
        axis=1) * SV).astype(E4M3)

    in_maps = []
    for c in range(N_CORES):
        xc = inputs[c * BL:(c + 1) * BL].reshape(BN, D)
        x0T = np.ascontiguousarray(xc.T)
        x8T = (x0T * SX).astype(E4M3)
        in_maps.append({
            "x0T": x0T,
            "x8T": x8T,
            "v0": v0,
            "v8": v8,
            "s8": s8,
            "bias": biases,
        })
    return in_maps


def kernel(inputs, adj, weights, biases):
    nc = _get_program()
    in_maps = make_in_maps(inputs, adj, weights, biases)
    res = run_bass_kernel_spmd(nc, in_maps, list(range(N_CORES)))
    out = np.concatenate(
        [res.results[c]["out"].reshape(BL, N, D) for c in range(N_CORES)],
        axis=0)
    return out


# revision 16
# speedup vs baseline: 1.5578x; 1.0046x over previous
"""DGCN diffusion-graph-conv kernel for 8 Trainium2 NeuronCores.

Math (per the reference):
    support S = D^-1/2 (adj+I)^T D^-1/2  with D = diag(rowsum(adj+I))
    x_m = T_m(S) x0  (Chebyshev recurrence, K=3 -> m=0..3)
    out = sum_m x_m @ W_m + bias

Strategy (data-parallel over batch, 4 batches/core, mixed precision):
    Fold Chebyshev coefficients into the weights:
        V0 = W0 - W2, V1 = W1 - 3*W3, V2 = 2*W2, V3 = 4*W3
        U_m = x0 @ V_m     (projection; contracts feature dim d)
        out = U0 + S*(U1 + S*(U2 + S*U3)) + bias   (Horner; contracts nodes)
    U0 feeds the output undamped -> computed in fp32r (full precision).
    U1..U3 and all S-multiplies are damped by the contractive support
    spectrum -> computed in fp8 e4m3 with DoubleRow matmuls (2x PE rate).
    Power-of-2 scales keep fp8 operands in the normal range:
        x*8, V_m*32 (m>=1), S*2^14, h (Horner state) carried *8.
    Host precomputes S, the V combos, transposes and fp8 quantization.
    Measured end-to-end rel err ~8e-3 (gate 2e-2).
"""

import numpy as np
import ml_dtypes

import concourse.bacc as bacc
import concourse.tile as tile
import concourse.mybir as mybir
from concourse.bass_utils import run_bass_kernel_spmd

F32 = mybir.dt.float32
F32R = mybir.dt.float32r
BF16 = mybir.dt.bfloat16
F8 = mybir.dt.float8e4
DR = mybir.MatmulPerfMode.DoubleRow
ALU = mybir.AluOpType
AFT = mybir.ActivationFunctionType
E4M3 = ml_dtypes.float8_e4m3

N_CORES = 8
B, N, D = 32, 512, 768
BL = B // N_CORES          # local batches per core = 4
BN = BL * N                # local rows = 2048
NT = N // 128              # 4 node tiles per batch
DT = D // 128              # 6 feature k-subtiles
PW = 1536                  # batch-pair column width (2*768)

SX = 8.0                   # x fp8 pre-scale
SV = 32.0                  # V1..V3 fp8 pre-scale
SS = float(2 ** 14)        # support fp8 pre-scale
SH = 8.0                   # Horner state carried *8 in fp8
C_U = 1.0 / (SX * SV / SH)         # psum(U123) -> 8*U_m      (= 1/32)
C_H = SH / (SS * SH)               # psum(S*h)  -> 8*(S@h)    (= 2^-14)
C_O = 1.0 / (SS * SH)              # psum(S*h1) -> S@h1       (= 2^-17)


def _build_program():
    nc = bacc.Bacc("TRN2", target_bir_lowering=False, debug=False,
                   num_devices=N_CORES)
    # All inputs host-permuted to the exact SBUF tile layout
    # ([partition, ...free]) so every DMA moves full contiguous lines.
    x0R_d = nc.dram_tensor("x0R", [BN // 128, 128, DT, 128], F32,
                           kind="ExternalInput").ap()
    x8P_d = nc.dram_tensor("x8P", [128, DT, BN], F8,
                           kind="ExternalInput").ap()
    v0P_d = nc.dram_tensor("v0P", [128, DT, D], F32,
                           kind="ExternalInput").ap()
    v8P_d = nc.dram_tensor("v8P", [128, DT, 3 * D], F8,
                           kind="ExternalInput").ap()
    s8P_d = nc.dram_tensor("s8P", [128, NT, N], F8,
                           kind="ExternalInput").ap()
    bias_d = nc.dram_tensor("bias", [D], F32, kind="ExternalInput").ap()
    out_d = nc.dram_tensor("out", [BN, D], F32, kind="ExternalOutput").ap()

    with tile.TileContext(nc) as tc:
        with (
            tc.tile_pool(name="const", bufs=1) as constp,
            tc.tile_pool(name="xs", bufs=3) as xsp,
            tc.tile_pool(name="ut", bufs=4) as utp,
            tc.tile_pool(name="u0t", bufs=2) as u0p,
            tc.tile_pool(name="h8t", bufs=3) as h8p,
            tc.tile_pool(name="ost", bufs=3) as ostp,
            tc.tile_pool(name="psU0", bufs=2, space="PSUM") as psU0,
            tc.tile_pool(name="psU", bufs=3, space="PSUM") as psU,
            tc.tile_pool(name="psH", bufs=3, space="PSUM") as psH,
        ):
            # ---- persistent inputs (spread across engine DMA queues so
            # the first projection matmuls aren't queued behind bulk) ----
            V0s = constp.tile([128, DT, D], F32R, name="V0s")
            X8 = constp.tile([128, DT, BN], F8, name="X8")
            nc.scalar.dma_start(X8[:], x8P_d[:, :, :])
            biasb = constp.tile([128, D], F32, name="biasb")
            nc.gpsimd.dma_start(
                biasb[:], bias_d.unsqueeze(0).broadcast_to([128, D]))
            V8s = constp.tile([128, DT, 3 * D], F8, name="V8s")
            nc.gpsimd.dma_start(V8s[:], v8P_d[:, :, :])
            S8s = constp.tile([128, NT, N], F8, name="S8s")
            nc.gpsimd.dma_start(S8s[:], s8P_d[:, :, :])

            def proj_u0(pair):
                """U0 (+bias) for batch pair, fp32r; result -> U0b fp32."""
                u0b = u0p.tile([128, NT, PW], F32, name=f"u0b{pair}",
                               tag="u0")
                for bi in range(2):
                    b = 2 * pair + bi
                    for nt in range(NT):
                        rt = b * NT + nt
                        xt = xsp.tile([128, DT, 128], F32R,
                                      name=f"xt{rt}", tag="xt")
                        xeng = (nc.sync, nc.scalar, nc.gpsimd)[rt % 3]
                        xeng.dma_start(
                            xt[:], x0R_d[rt, :, :, :].bitcast(F32R))
                        if pair == 0 and bi == 0 and nt == 0:
                            # V0 chunks follow the first stationary tile so
                            # the first projection group starts early
                            for t in range(DT):
                                nc.sync.dma_start(
                                    V0s[:, t, :],
                                    v0P_d[:, t, :].bitcast(F32R))
                        ps = psU0.tile([128, 512], F32, name=f"pA{rt}",
                                       tag="ps")
                        ps2 = psU0.tile([128, 512], F32, name=f"pB{rt}",
                                        tag="ps")
                        # t outer: both column groups share each stationary
                        for t in range(DT):
                            nc.tensor.matmul(
                                ps[:], xt[:, t, :], V0s[:, t, 0:512],
                                start=(t == 0), stop=(t == DT - 1))
                            nc.tensor.matmul(
                                ps2[:, 0:256], xt[:, t, :], V0s[:, t, 512:D],
                                start=(t == 0), stop=(t == DT - 1))
                        c0 = bi * D
                        nc.vector.tensor_add(
                            u0b[:, nt, c0:c0 + 512], ps[:], biasb[:, 0:512])
                        nc.vector.tensor_add(
                            u0b[:, nt, c0 + 512:c0 + D], ps2[:, 0:256],
                            biasb[:, 512:D])
                return u0b

            def proj_u123(pair):
                """U1..U3 for batch pair, fp8 DoubleRow.

                U1,U2 -> bf16 (*8); U3 -> fp8 (*8) straight into h3."""
                u = {m: utp.tile([128, NT, PW], BF16, name=f"u{pair}_{m}",
                                 tag="u")
                     for m in (1, 2)}
                h3 = h8p.tile([128, NT, PW], F8, name=f"h3_{pair}", tag="h8")
                for bi in range(2):
                    b = 2 * pair + bi
                    for nt in range(NT):
                        rt = b * NT + nt
                        c0 = bi * D
                        for m in (1, 2, 3):
                            vb = (m - 1) * D
                            ps = psU.tile([128, 512], F32,
                                          name=f"pU{rt}_{m}", tag="ps")
                            ps2 = psU.tile([128, 512], F32,
                                           name=f"pV{rt}_{m}", tag="ps")
                            # t outer: both column groups share the stationary
                            for t in range(DT // 2):
                                xs = X8[:, 2 * t:2 * t + 2,
                                        rt * 128:(rt + 1) * 128]
                                nc.tensor.matmul(
                                    ps[:], xs,
                                    V8s[:, 2 * t:2 * t + 2, vb:vb + 512],
                                    start=(t == 0), stop=(t == 2),
                                    perf_mode=DR)
                                nc.tensor.matmul(
                                    ps2[:, 0:256], xs,
                                    V8s[:, 2 * t:2 * t + 2, vb + 512:vb + D],
                                    start=(t == 0), stop=(t == 2),
                                    perf_mode=DR)
                            dst = u[m] if m != 3 else h3
                            nc.scalar.activation(
                                dst[:, nt, c0:c0 + 512], ps[:],
                                AFT.Copy, scale=C_U)
                            nc.scalar.activation(
                                dst[:, nt, c0 + 512:c0 + D], ps2[:, 0:256],
                                AFT.Copy, scale=C_U)
                return u, h3

            def horner_step(pair, h_cur, u, s, u0b):
                """One Horner step for a batch pair. s=2,1 -> new fp8 h;
                s=0 -> final output + DMA."""
                h_next = None
                if s > 0:
                    h_next = h8p.tile([128, NT, PW], F8,
                                      name=f"h{s}_{pair}", tag="h8")
                for nt in range(NT):
                    ot = None
                    if s == 0:
                        ot = ostp.tile([128, PW], F32, name=f"o{pair}_{nt}",
                                       tag="ost")
                    pss = [psH.tile([128, 512], F32,
                                    name=f"pH{pair}{s}{nt}{ck}", tag="ps")
                           for ck in range(PW // 512)]
                    # t outer: all three column chunks share the stationary
                    for t in range(NT // 2):
                        st = S8s[:, 2 * t:2 * t + 2, nt * 128:(nt + 1) * 128]
                        for ck in range(PW // 512):
                            nc.tensor.matmul(
                                pss[ck][:], st,
                                h_cur[:, 2 * t:2 * t + 2,
                                      ck * 512:(ck + 1) * 512],
                                start=(t == 0), stop=(t == 1),
                                perf_mode=DR)
                    for ck in range(PW // 512):
                        ps = pss[ck]
                        if s > 0:
                            nc.vector.scalar_tensor_tensor(
                                h_next[:, nt, ck * 512:(ck + 1) * 512],
                                ps[:], C_H,
                                u[s][:, nt, ck * 512:(ck + 1) * 512],
                                ALU.mult, ALU.add)
                        else:
                            nc.vector.scalar_tensor_tensor(
                                ot[:, ck * 512:(ck + 1) * 512],
                                ps[:], C_O,
                                u0b[:, nt, ck * 512:(ck + 1) * 512],
                                ALU.mult, ALU.add)
                    if s == 0:
                        for bi in range(2):
                            b = 2 * pair + bi
                            r0 = b * N + nt * 128
                            eng = (nc.gpsimd, nc.sync, nc.scalar)[
                                (2 * nt + bi) % 3]
                            eng.dma_start(
                                out_d[r0:r0 + 128, :],
                                ot[:, bi * D:(bi + 1) * D])
                return h_next

            # ---- schedule ----
            u0b_0 = proj_u0(0)
            u_0, h3_0 = proj_u123(0)

            h2_0 = horner_step(0, h3_0, u_0, 2, u0b_0)
            u0b_1 = proj_u0(1)
            h1_0 = horner_step(0, h2_0, u_0, 1, u0b_0)
            u_1, h3_1 = proj_u123(1)
            horner_step(0, h1_0, u_0, 0, u0b_0)

            h2_1 = horner_step(1, h3_1, u_1, 2, u0b_1)
            h1_1 = horner_step(1, h2_1, u_1, 1, u0b_1)
            horner_step(1, h1_1, u_1, 0, u0b_1)
    nc.compile()
    return nc


_CACHE = {}


def _get_program():
    if "nc" not in _CACHE:
        _CACHE["nc"] = _build_program()
    return _CACHE["nc"]


def make_in_maps(inputs, adj, weights, biases):
    inputs = np.ascontiguousarray(inputs, dtype=np.float32)
    adj = np.ascontiguousarray(adj, dtype=np.float32)
    weights = np.ascontiguousarray(weights, dtype=np.float32)
    biases = np.ascontiguousarray(biases, dtype=np.float32)
    assert inputs.shape == (B, N, D)
    assert adj.shape == (N, N)
    assert weights.shape == (D * 4, D)
    assert biases.shape == (D,)

    def perm(a, kt):
        # [kt*128, F] -> [128, kt, F] partition-major tile image
        F = a.shape[1]
        return np.ascontiguousarray(
            a.reshape(kt, 128, F).transpose(1, 0, 2))

    # support S = D^-1/2 (adj+I)^T D^-1/2, transposed for the lhsT layout
    m = adj + np.eye(N, dtype=np.float32)
    dd = m.sum(axis=1) ** -0.5
    S = ((m * dd[None, :]).T * dd[None, :]).astype(np.float32)
    s8P = perm(np.ascontiguousarray(S.T * SS).astype(E4M3), NT)

    W4 = weights.reshape(D, 4, D)
    v0P = perm(np.ascontiguousarray(W4[:, 0] - W4[:, 2]), DT)
    v8P = perm(np.ascontiguousarray(np.concatenate(
        [W4[:, 1] - 3.0 * W4[:, 3], 2.0 * W4[:, 2], 4.0 * W4[:, 3]],
        axis=1) * SV).astype(E4M3), DT)

    in_maps = []
    for c in range(N_CORES):
        xc = inputs[c * BL:(c + 1) * BL].reshape(BN, D)
        # x0R[rt, p, t, r] = xc[rt*128+r, t*128+p]
        x0R = np.ascontiguousarray(
            xc.reshape(BN // 128, 128, DT, 128).transpose(0, 3, 2, 1))
        x8P = perm((xc.T * SX).astype(E4M3), DT)
        in_maps.append({
            "x0R": x0R,
            "x8P": x8P,
            "v0P": v0P,
            "v8P": v8P,
            "s8P": s8P,
            "bias": biases,
        })
    return in_maps


def kernel(inputs, adj, weights, biases):
    nc = _get_program()
    in_maps = make_in_maps(inputs, adj, weights, biases)
    res = run_bass_kernel_spmd(nc, in_maps, list(range(N_CORES)))
    out = np.concatenate(
        [res.results[c]["out"].reshape(BL, N, D) for c in range(N_CORES)],
        axis=0)
    return out
